# revision 7
# baseline (speedup 1.0000x reference)
"""Multi-head self-attention (b=2, n=2048, emb=1024, heads=16) on 8 trn2 cores.

Sharding: core c = (b, hg) with b = c // 4, hg = c % 4. Data parallel over
batch, tensor parallel over head-groups (4 heads / 256 emb-cols per core).
Each core computes Q/K/V projections for its heads, full attention for its
heads, and a partial output projection ctx_hg @ Wo[:, hg_slice].T of shape
[2048, 1024]. The host sums the 4 partials per batch (Megatron row-parallel
reduce done on host) and adds the rank-1 bias term bv @ Wo.T + bo.

Device layout notes:
- x^T [emb, n] fp16 (4MB = 32KB/partition) is loaded ONCE and stays resident
  in SBUF; both head-pairs' projections read it, so there is no mid-kernel
  x DMA traffic and no DMA wait can head-of-line-block the in-order PE queue.
- DMA issue is split across the Sync and GpSimd queues so sequencer DGE time
  doesn't serialize the startup (weights on Sync, x + output stores on
  GpSimd).
- Q^T, K^T are produced in [dq, n] layout (dq = head-major), V in natural
  [n, dv] layout augmented with a ones column per head -> the ctx matmul
  ctxT[65, nq] = V_aug^T @ E^T produces softmax row-sums in row 64 for free.
- Attention runs on 512-wide nq chunks: S matmuls, 1024-wide exps straight
  out of PSUM, and ctx matmuls all stream 512 columns, halving the PE
  instruction count (and its stall overhead) vs 256-wide chunks.
- Softmax normalization: row-sum staged to partition 0 (DVE), broadcast
  (GpSimd), reciprocal_approx_fast (single custom-DVE op, ~5x faster than
  InstReciprocal; denominators are strictly positive sums of exps so the
  approx's edge cases can't occur), then one fused multiply straight out of
  the ctx PSUM bank into ctxT fp16.
- Output projection accumulates both head-pair contributions into one PSUM
  group (2 matmuls back-to-back), needing a single PSUM->SBUF copy and no
  SBUF accumulator array.
- All matmuls run in float16 (1 cyc/col on PE; 10-bit mantissa keeps the
  overall error ~7e-4 scale-relative, validated vs fp32).
- q/k biases are added on-device (fused into the PSUM->SBUF copy);
  v/o biases are exactly the rank-1 host-side term above.
"""

import os
import sys

for _p in ("/opt/trn_rl_repo", "/root/.axon_site/_ro/trn_rl_repo"):
    if os.path.isdir(_p) and _p not in sys.path:
        sys.path.append(_p)

import numpy as np

import concourse.bass as bass  # noqa: F401  (engine types pulled via nc)
import concourse.mybir as mybir
import concourse.tile as tile
from concourse import bacc
from concourse.bass_utils import run_bass_kernel_spmd

B, N, EMB, HEADS, HD = 2, 2048, 1024, 16, 64
N_CORES = 8
TP = 4                      # head-group shards per batch
DQ = EMB // TP              # 256 emb-cols (4 heads) per core
SCALE = HD ** -0.5          # 0.125

F32 = mybir.dt.float32
F16 = mybir.dt.float16
FP = mybir.ActivationFunctionType

NQ = 512                    # nq chunk for projections / out-proj
NJ = N // NQ                # 4 nq chunks
NQA = 512                   # nq chunk for attention
NJA = N // NQA              # 4 attention nq chunks
NKC = 128                   # nk chunk (ctx contraction)
NT = N // NKC               # 16 nk chunks
KC = EMB // 128             # 8 e chunks
GK = 2                      # nk chunks per S-psum group (1024-wide exps)
NG = NT // GK               # 8 groups per (head, j)


def build_program():
    """Build + compile the single SPMD program all 8 cores run."""
    nc = bacc.Bacc("TRN2", target_bir_lowering=False, debug=False,
                   num_devices=N_CORES)

    xT = nc.dram_tensor("xT", [EMB, N], F16, kind="ExternalInput").ap()
    wqT = nc.dram_tensor("wqT", [EMB, DQ], F16, kind="ExternalInput").ap()
    wkT = nc.dram_tensor("wkT", [EMB, DQ], F16, kind="ExternalInput").ap()
    wvT = nc.dram_tensor("wvT", [EMB, DQ], F16, kind="ExternalInput").ap()
    woT = nc.dram_tensor("woT", [DQ, EMB], F16, kind="ExternalInput").ap()
    bqd = nc.dram_tensor("bq_s", [DQ], F32, kind="ExternalInput").ap()
    bkd = nc.dram_tensor("bk_s", [DQ], F32, kind="ExternalInput").ap()
    # fp16 partials: host sums 4 of them in fp32; quantization of the
    # partial (|.| ~ 1, ulp ~ 1e-3) adds ~1e-3 absmax-relative error --
    # well under the 2e-2 gate -- and halves the store traffic + tail.
    out_part = nc.dram_tensor("out_part", [N, EMB], F16,
                              kind="ExternalOutput").ap()

    with tile.TileContext(nc) as tc:
        with (
            tc.tile_pool(name="const", bufs=1) as const,
            tc.tile_pool(name="persist", bufs=1) as persist,
            tc.tile_pool(name="epool", bufs=3) as epool,
            tc.tile_pool(name="npool", bufs=2) as npool,
            tc.tile_pool(name="opool", bufs=4) as opool,
            # PSUM static budget (8 banks): pp 2 + s0 2 + s1 2 + c0 1 + c1 1
            tc.tile_pool(name="ppool", bufs=2, space="PSUM") as ppool,
            tc.tile_pool(name="spool", bufs=1, space="PSUM") as spool,
            tc.tile_pool(name="cpool", bufs=1, space="PSUM") as cpool,
        ):
            # ---- constants + resident x ----
            # first projection group needs wk chunks + x n-slice 0 only; those
            # DMAs go first, fine-grained, spread across the Sync / GpSimd /
            # Scalar queues so no single sequencer's DGE time serializes the
            # startup. x n-slice 0 is 8 small chunk DMAs (fast first arrival);
            # slices 1-3 ride 8 bigger strided slab DMAs.
            wq_sb = const.tile([128, KC, DQ], F16, tag="wq")
            wk_sb = const.tile([128, KC, DQ], F16, tag="wk")
            wv_sb = const.tile([128, KC, DQ], F16, tag="wv")
            x_sb = const.tile([128, KC, N], F16, tag="x")
            xTr = xT.rearrange("(k p) n -> k p n", p=128)
            bk_sb = const.tile([128, 2], F32, tag="bk")
            nc.sync.dma_start(out=bk_sb, in_=bkd.rearrange("(m p) -> p m", p=128))
            for k in range(KC):
                nc.sync.dma_start(out=wk_sb[:, k, :], in_=wkT.rearrange(
                    "(k p) d -> k p d", p=128)[k])
                nc.gpsimd.dma_start(out=x_sb[:, k, 0:NQ], in_=xTr[k, :, 0:NQ])
            for k in range(KC):
                eng = nc.sync if k % 2 == 0 else nc.gpsimd
                eng.dma_start(out=x_sb[:, k, NQ:N], in_=xTr[k, :, NQ:N])
                nc.scalar.dma_start(out=wv_sb[:, k, :], in_=wvT.rearrange(
                    "(k p) d -> k p d", p=128)[k])
            for k in range(KC):
                nc.scalar.dma_start(out=wq_sb[:, k, :], in_=wqT.rearrange(
                    "(k p) d -> k p d", p=128)[k])
            bq_sb = const.tile([128, 2], F32, tag="bq")
            nc.scalar.dma_start(out=bq_sb, in_=bqd.rearrange("(m p) -> p m", p=128))
            # wo is needed only by the out-projection (~100us in) — deferred
            # into the filler stream to keep startup queues clear
            wo_sb = const.tile([128, 2, EMB], F16, tag="wo")

            # ---- persistent activations ----
            qT = [persist.tile([128, N], F16, tag=f"qT{p}", name=f"qT{p}") for p in range(2)]
            kT = [persist.tile([128, N], F16, tag=f"kT{p}", name=f"kT{p}") for p in range(2)]
            ctxT = [persist.tile([128, N], F16, tag=f"ctxT{p}", name=f"ctxT{p}") for p in range(2)]
            # V for all 4 local heads: [nk-part, t, head*65 + (0:64 | ones)]
            v_all = persist.tile([128, NT, 4 * (HD + 1)], F16, tag="v")
            for h in range(4):
                nc.vector.memset(v_all[:, :, h * 65 + 64], 1.0)

            add, mult = mybir.AluOpType.add, mybir.AluOpType.mult

            # ---- projection building blocks ----
            # Each is one PSUM accumulation group on the double-buffered pp
            # tag, small enough to slot between attention groups.
            def kq_group(p, n, wsb, bsb, dst):
                ps = ppool.tile([128, NQ], F32, tag="pp", name="kqp")
                for k in range(KC):
                    nc.tensor.matmul(
                        ps, wsb[:, k, p * 128:(p + 1) * 128],
                        x_sb[:, k, n * NQ:(n + 1) * NQ],
                        start=(k == 0), stop=(k == KC - 1))
                nc.vector.tensor_tensor(
                    out=dst[p][:, n * NQ:(n + 1) * NQ], in0=ps,
                    in1=bsb[:, p:p + 1].broadcast_to([128, NQ]), op=add)

            def v_group(p, n, tl):
                t = n * 4 + tl
                ps = ppool.tile([128, NQ], F32, tag="pp", name="vp")
                for k in range(KC):
                    nc.tensor.matmul(
                        ps[:, 0:128],
                        x_sb[:, k, t * 128:(t + 1) * 128],
                        wv_sb[:, k, p * 128:(p + 1) * 128],
                        start=(k == 0), stop=(k == KC - 1))
                vv = v_all[:, t, :].rearrange("p (h c) -> p h c", c=65)
                nc.vector.tensor_copy(
                    out=vv[:, 2 * p:2 * p + 2, 0:64],
                    in_=ps[:, 0:128].rearrange("p (h c) -> p h c", c=64))

            def proj_fillers(p, defer_q=False):
                # K for all n, V for all t, then Q; when defer_q, only Q(n=0)
                # is emitted inline (attention j=0 needs just that slice) and
                # Q(1..3) are returned separately as fillers.
                out = []
                for n in range(NJ):
                    out.append(lambda p=p, n=n: kq_group(p, n, wk_sb, bk_sb, kT))
                for n in range(NJ):
                    for tl in range(4):
                        out.append(lambda p=p, n=n, tl=tl: v_group(p, n, tl))
                qs = [lambda p=p, n=n: kq_group(p, n, wq_sb, bq_sb, qT)
                      for n in range(NJ)]
                if defer_q:
                    return out + qs[:1], qs[1:]
                return out + qs, []

            # pair-0 K/V/Q0 run up front (serial ACT-idle prefix; attention
            # needs all of kT0/v + qT0 j-slice before it can start)
            head, q_rest = proj_fillers(0, defer_q=True)
            for f in head:
                f()

            # ---- out-projection ----
            # one (m, eo) parcel: both head-pair passes accumulate into a
            # single pp-tag PSUM group, then one copy to SBUF + store.
            def out_proj_parcel(m, eo):
                po = ppool.tile([128, NQ], F32, tag="pp", name="po")
                for kp in range(2):
                    nc.tensor.matmul(
                        po, ctxT[kp][:, m * 128:(m + 1) * 128],
                        wo_sb[:, kp, eo * NQ:(eo + 1) * NQ],
                        start=(kp == 0), stop=(kp == 1))
                o = opool.tile([128, NQ], F16, tag="o", name="o")
                nc.vector.tensor_copy(o, po)
                nc.gpsimd.dma_start(
                    out=out_part[m * 128:(m + 1) * 128, eo * NQ:(eo + 1) * NQ],
                    in_=o)

            # ---- attention (per head-pair p, nq chunk j of 512) ----
            # Software-pipelined: ctx matmuls for work item u are emitted
            # after the S/exp of item u+1, so PE always has ready work while
            # ACT streams 1024-wide exps; heads alternate as the natural PSUM
            # ping-pong for the S tiles. Filler parcels (projections, wo DMA,
            # out-proj) are popped between work items, paced to spread over
            # the remaining j windows.
            from collections import deque
            fillers = deque()

            for p in range(2):
                if p == 0:
                    fillers.extend(q_rest)
                    fillers.append(lambda: nc.sync.dma_start(
                        out=wo_sb,
                        in_=woT.rearrange("(k p) e -> p k e", p=128)))
                    nf, _ = proj_fillers(1)
                    fillers.extend(nf)
                for j in range(NJA):
                    cps = [cpool.tile([HD + 1, NQA], F32, tag=f"c{h}",
                                      name=f"c{h}") for h in range(2)]

                    def s_mms(g, h):
                        lo = 64 * h
                        sp = spool.tile([128, GK, NQA], F32,
                                        tag=f"s{h}", name=f"s{h}")
                        for i, t in enumerate(g):
                            nc.tensor.matmul(
                                sp[:, i, :],
                                kT[p][lo:lo + 64, t * 128:(t + 1) * 128],
                                qT[p][lo:lo + 64, j * NQA:(j + 1) * NQA],
                                start=True, stop=True)
                        return sp

                    def exp_act(sp, g, h):
                        e = epool.tile([128, GK, NQA], F16,
                                       tag=f"e{h}", name=f"e{h}")
                        nc.scalar.activation(e, sp, FP.Exp, scale=SCALE)
                        return e

                    def ctx_mms(e, g, h):
                        hloc = 2 * p + h
                        for i, t in enumerate(g):
                            nc.tensor.matmul(
                                cps[h],
                                v_all[:, t, hloc * 65:(hloc + 1) * 65],
                                e[:, i, :],
                                start=(t == 0), stop=(t == NT - 1))

                    work = []
                    for gi in range(NG):
                        for h in range(2):
                            work.append((tuple(range(gi * GK, (gi + 1) * GK)), h))
                    n_pop = -(-len(fillers) // (NJA - j))  # ceil
                    pend = deque()
                    for wi, (g, h) in enumerate(work):
                        sp = s_mms(g, h)
                        pend.append((exp_act(sp, g, h), g, h))
                        # ctx trails by TWO work items so its exp input is
                        # safely done and the in-order PE queue never stalls
                        # on ACT
                        if len(pend) > 2:
                            ctx_mms(*pend.popleft())
                        if fillers and wi < n_pop:
                            fillers.popleft()()
                    while pend:
                        ctx_mms(*pend.popleft())
                    for _ in range(len(work), n_pop):
                        if fillers:
                            fillers.popleft()()

                    # normalize: ctx^T[0:64] * (1 / rowsum); rowsum in row 64.
                    # Two quick DVE copies (rowsum row -> partition 0 for
                    # partition_broadcast, ctx body -> SBUF) release the ctx
                    # PSUM bank ~1.5us after the last ctx matmul so the next
                    # j's ctx accumulation never stalls on it; the slow-ish
                    # broadcast + reciprocal then run off the critical path.
                    for h in range(2):
                        rs = npool.tile([1, NQA], F32, tag="rs", name="rs")
                        nc.vector.tensor_copy(rs, cps[h][64:65, :])
                        cs = npool.tile([64, NQA], F32, tag="cs", name="cs")
                        nc.vector.tensor_copy(cs, cps[h][0:64, :])
                        rb = npool.tile([64, NQA], F32, tag="rb", name="rb")
                        nc.gpsimd.partition_broadcast(rb, rs)
                        rc = npool.tile([64, NQA], F32, tag="rc", name="rc")
                        nc.vector.reciprocal_approx_fast(out=rc, in_=rb)
                        nc.vector.tensor_tensor(
                            out=ctxT[p][h * 64:(h + 1) * 64,
                                        j * NQA:(j + 1) * NQA],
                            in0=cs, in1=rc, op=mult)
                    if p == 1:
                        # ctxT1 columns for this j are final -> out-proj
                        # parcels for the covered m-chunks can run
                        for m in range(4 * j, 4 * j + 4):
                            for eo in range(2):
                                fillers.append(
                                    lambda m=m, eo=eo: out_proj_parcel(m, eo))
            while fillers:
                fillers.popleft()()

    nc.compile()
    return nc


_NC_CACHE = {}


def _get_program():
    if "nc" not in _NC_CACHE:
        _NC_CACHE["nc"] = build_program()
    return _NC_CACHE["nc"]


def make_in_maps(x, Wq, bq, Wk, bk, Wv, bv, Wo, bo):
    x = np.asarray(x)
    xTs = [np.ascontiguousarray(x[b].T.astype(np.float16)) for b in range(B)]
    in_maps = []
    for c in range(N_CORES):
        b, hg = divmod(c, TP)
        sl = slice(hg * DQ, (hg + 1) * DQ)
        in_maps.append({
            "xT": xTs[b],
            "wqT": np.ascontiguousarray(np.asarray(Wq, np.float16)[sl, :].T),
            "wkT": np.ascontiguousarray(np.asarray(Wk, np.float16)[sl, :].T),
            "wvT": np.ascontiguousarray(np.asarray(Wv, np.float16)[sl, :].T),
            "woT": np.ascontiguousarray(np.asarray(Wo, np.float16)[:, sl].T),
            "bq_s": np.ascontiguousarray(np.asarray(bq, np.float32)[sl]),
            "bk_s": np.ascontiguousarray(np.asarray(bk, np.float32)[sl]),
        })
    return in_maps


def assemble_output(results, Wv_bias_term):
    out = np.empty((B, N, EMB), np.float32)
    for b in range(B):
        acc = results[b * TP]["out_part"].astype(np.float32)
        for g in range(1, TP):
            acc = acc + results[b * TP + g]["out_part"]
        out[b] = acc + Wv_bias_term
    return out


def kernel(x, Wq, bq, Wk, bk, Wv, bv, Wo, bo):
    nc = _get_program()
    in_maps = make_in_maps(x, Wq, bq, Wk, bk, Wv, bv, Wo, bo)
    res = run_bass_kernel_spmd(nc, in_maps, list(range(N_CORES)))
    bias_term = (np.asarray(bv, np.float32) @ np.asarray(Wo, np.float32).T
                 + np.asarray(bo, np.float32))
    return assemble_output(res.results, bias_term)


# revision 11
# speedup vs baseline: 1.1714x; 1.1714x over previous
"""Multi-head self-attention (b=2, n=2048, emb=1024, heads=16) on 8 trn2 cores.

Sharding: core c = (b, hg) with b = c // 4, hg = c % 4. Data parallel over
batch, tensor parallel over head-groups (4 heads / 256 emb-cols per core).
Each core computes Q/K/V projections for its heads, full attention for its
heads, and a partial output projection ctx_hg @ Wo[:, hg_slice].T of shape
[2048, 1024]. The host sums the 4 partials per batch (Megatron row-parallel
reduce done on host) and adds the rank-1 bias term bv @ Wo.T + bo.

Device layout notes:
- x^T [emb, n] fp16 (4MB = 32KB/partition) is loaded ONCE and stays resident
  in SBUF; both head-pairs' projections read it, so there is no mid-kernel
  x DMA traffic and no DMA wait can head-of-line-block the in-order PE queue.
- DMA issue is split across the Sync and GpSimd queues so sequencer DGE time
  doesn't serialize the startup (weights on Sync, x + output stores on
  GpSimd).
- Q^T, K^T are produced in [dq, n] layout (dq = head-major), V in natural
  [n, dv] layout augmented with a ones column per head -> the ctx matmul
  ctxT[65, nq] = V_aug^T @ E^T produces softmax row-sums in row 64 for free.
- Attention runs on 512-wide nq chunks: S matmuls, 1024-wide exps straight
  out of PSUM, and ctx matmuls all stream 512 columns, halving the PE
  instruction count (and its stall overhead) vs 256-wide chunks.
- Softmax normalization: row-sum staged to partition 0 (DVE), broadcast
  (GpSimd), reciprocal_approx_fast (single custom-DVE op, ~5x faster than
  InstReciprocal; denominators are strictly positive sums of exps so the
  approx's edge cases can't occur), then one fused multiply straight out of
  the ctx PSUM bank into ctxT fp16.
- Output projection accumulates both head-pair contributions into one PSUM
  group (2 matmuls back-to-back), needing a single PSUM->SBUF copy and no
  SBUF accumulator array.
- All matmuls run in float16 (1 cyc/col on PE; 10-bit mantissa keeps the
  overall error ~7e-4 scale-relative, validated vs fp32).
- q/k biases are added on-device (fused into the PSUM->SBUF copy);
  v/o biases are exactly the rank-1 host-side term above.
"""

import os
import sys

for _p in ("/opt/trn_rl_repo", "/root/.axon_site/_ro/trn_rl_repo"):
    if os.path.isdir(_p) and _p not in sys.path:
        sys.path.append(_p)

import numpy as np

import concourse.bass as bass  # noqa: F401  (engine types pulled via nc)
import concourse.mybir as mybir
import concourse.tile as tile
from concourse import bacc
from concourse.bass_utils import run_bass_kernel_spmd

B, N, EMB, HEADS, HD = 2, 2048, 1024, 16, 64
N_CORES = 8
TP = 4                      # head-group shards per batch
DQ = EMB // TP              # 256 emb-cols (4 heads) per core
SCALE = HD ** -0.5          # 0.125

F32 = mybir.dt.float32
F16 = mybir.dt.float16
FP = mybir.ActivationFunctionType

NQ = 512                    # nq chunk for projections / out-proj
NJ = N // NQ                # 4 nq chunks
NQA = 512                   # nq chunk for attention
NJA = N // NQA              # 4 attention nq chunks
NKC = 128                   # nk chunk (ctx contraction)
NT = N // NKC               # 16 nk chunks
KC = EMB // 128             # 8 e chunks
GK = 2                      # nk chunks per S-psum group (1024-wide exps)
NG = NT // GK               # 8 groups per (head, j)


def build_program():
    """Build + compile the single SPMD program all 8 cores run."""
    nc = bacc.Bacc("TRN2", target_bir_lowering=False, debug=False,
                   num_devices=N_CORES)

    xT = nc.dram_tensor("xT", [EMB, N], F16, kind="ExternalInput").ap()
    wqT = nc.dram_tensor("wqT", [EMB, DQ], F16, kind="ExternalInput").ap()
    wkT = nc.dram_tensor("wkT", [EMB, DQ], F16, kind="ExternalInput").ap()
    wvT = nc.dram_tensor("wvT", [EMB, DQ], F16, kind="ExternalInput").ap()
    woT = nc.dram_tensor("woT", [DQ, EMB], F16, kind="ExternalInput").ap()
    bqd = nc.dram_tensor("bq_s", [DQ], F32, kind="ExternalInput").ap()
    bkd = nc.dram_tensor("bk_s", [DQ], F32, kind="ExternalInput").ap()
    # fp16 partials: host sums 4 of them in fp32; quantization of the
    # partial (|.| ~ 1, ulp ~ 1e-3) adds ~1e-3 absmax-relative error --
    # well under the 2e-2 gate -- and halves the store traffic + tail.
    out_part = nc.dram_tensor("out_part", [N, EMB], F16,
                              kind="ExternalOutput").ap()

    with tile.TileContext(nc) as tc:
        with (
            tc.tile_pool(name="const", bufs=1) as const,
            tc.tile_pool(name="persist", bufs=1) as persist,
            tc.tile_pool(name="epool", bufs=3) as epool,
            tc.tile_pool(name="npool", bufs=2) as npool,
            tc.tile_pool(name="opool", bufs=4) as opool,
            # PSUM static budget (8 banks): pp 2 + s0 2 + s1 2 + c0 1 + c1 1
            tc.tile_pool(name="ppool", bufs=2, space="PSUM") as ppool,
            tc.tile_pool(name="spool", bufs=1, space="PSUM") as spool,
            tc.tile_pool(name="cpool", bufs=1, space="PSUM") as cpool,
        ):
            # ---- constants + resident x ----
            # first projection group needs wk chunks + x n-slice 0 only; those
            # DMAs go first, fine-grained, spread across the Sync / GpSimd /
            # Scalar queues so no single sequencer's DGE time serializes the
            # startup. x n-slice 0 is 8 small chunk DMAs (fast first arrival);
            # slices 1-3 ride 8 bigger strided slab DMAs.
            wq_sb = const.tile([128, KC, DQ], F16, tag="wq")
            wk_sb = const.tile([128, KC, DQ], F16, tag="wk")
            wv_sb = const.tile([128, KC, DQ], F16, tag="wv")
            x_sb = const.tile([128, KC, N], F16, tag="x")
            xTr = xT.rearrange("(k p) n -> k p n", p=128)
            bk_sb = const.tile([128, 2], F32, tag="bk")
            nc.sync.dma_start(out=bk_sb, in_=bkd.rearrange("(m p) -> p m", p=128))
            # x chunks arrive in exactly the order the projection prefix
            # consumes them (n-major), alternating n-slices between the
            # GpSimd and Sync queues; weights ride the Sync/Vector/Scalar
            # queues so no single sequencer serializes the startup.
            for k in range(KC):
                nc.sync.dma_start(out=wk_sb[:, k, :], in_=wkT.rearrange(
                    "(k p) d -> k p d", p=128)[k])
                nc.gpsimd.dma_start(out=x_sb[:, k, 0:NQ], in_=xTr[k, :, 0:NQ])
            for n in range(1, NJ):
                eng = nc.sync if n % 2 == 1 else nc.gpsimd
                for k in range(KC):
                    eng.dma_start(out=x_sb[:, k, n * NQ:(n + 1) * NQ],
                                  in_=xTr[k, :, n * NQ:(n + 1) * NQ])
            for k in range(KC):
                nc.scalar.dma_start(out=wv_sb[:, k, :], in_=wvT.rearrange(
                    "(k p) d -> k p d", p=128)[k])
            for k in range(KC):
                nc.scalar.dma_start(out=wq_sb[:, k, :], in_=wqT.rearrange(
                    "(k p) d -> k p d", p=128)[k])
            bq_sb = const.tile([128, 2], F32, tag="bq")
            nc.scalar.dma_start(out=bq_sb, in_=bqd.rearrange("(m p) -> p m", p=128))
            # wo is needed only by the out-projection (~100us in) — deferred
            # into the filler stream to keep startup queues clear
            wo_sb = const.tile([128, 2, EMB], F16, tag="wo")

            # ---- persistent activations ----
            qT = [persist.tile([128, N], F16, tag=f"qT{p}", name=f"qT{p}") for p in range(2)]
            kT = [persist.tile([128, N], F16, tag=f"kT{p}", name=f"kT{p}") for p in range(2)]
            ctxT = [persist.tile([128, N], F16, tag=f"ctxT{p}", name=f"ctxT{p}") for p in range(2)]
            # V for all 4 local heads: [nk-part, t, head*65 + (0:64 | ones)]
            v_all = persist.tile([128, NT, 4 * (HD + 1)], F16, tag="v")
            for h in range(4):
                nc.vector.memset(v_all[:, :, h * 65 + 64], 1.0)

            add, mult = mybir.AluOpType.add, mybir.AluOpType.mult

            # ---- projection building blocks ----
            # Each is one PSUM accumulation group on the double-buffered pp
            # tag, small enough to slot between attention groups.
            def kq_group(p, n, wsb, bsb, dst):
                ps = ppool.tile([128, NQ], F32, tag="pp", name="kqp")
                for k in range(KC):
                    nc.tensor.matmul(
                        ps, wsb[:, k, p * 128:(p + 1) * 128],
                        x_sb[:, k, n * NQ:(n + 1) * NQ],
                        start=(k == 0), stop=(k == KC - 1))
                nc.vector.tensor_tensor(
                    out=dst[p][:, n * NQ:(n + 1) * NQ], in0=ps,
                    in1=bsb[:, p:p + 1].broadcast_to([128, NQ]), op=add)

            def v_group(p, n, tl):
                t = n * 4 + tl
                ps = ppool.tile([128, NQ], F32, tag="pp", name="vp")
                for k in range(KC):
                    nc.tensor.matmul(
                        ps[:, 0:128],
                        x_sb[:, k, t * 128:(t + 1) * 128],
                        wv_sb[:, k, p * 128:(p + 1) * 128],
                        start=(k == 0), stop=(k == KC - 1))
                vv = v_all[:, t, :].rearrange("p (h c) -> p h c", c=65)
                nc.vector.tensor_copy(
                    out=vv[:, 2 * p:2 * p + 2, 0:64],
                    in_=ps[:, 0:128].rearrange("p (h c) -> p h c", c=64))

            def proj_fillers(p, defer_q=False):
                # K/V interleaved per n-slice (matches x DMA arrival order),
                # then Q; when defer_q, only Q(n=0) is emitted inline
                # (attention j=0 needs just that slice) and Q(1..3) are
                # returned separately as fillers.
                out = []
                for n in range(NJ):
                    out.append(lambda p=p, n=n: kq_group(p, n, wk_sb, bk_sb, kT))
                    for tl in range(4):
                        out.append(lambda p=p, n=n, tl=tl: v_group(p, n, tl))
                qs = [lambda p=p, n=n: kq_group(p, n, wq_sb, bq_sb, qT)
                      for n in range(NJ)]
                if defer_q:
                    return out + qs[:1], qs[1:]
                return out + qs, []

            # pair-0 K/V/Q0 run up front (serial ACT-idle prefix; attention
            # needs all of kT0/v + qT0 j-slice before it can start)
            head, q_rest = proj_fillers(0, defer_q=True)
            for f in head:
                f()

            # ---- out-projection ----
            # one (m, eo) parcel: both head-pair passes accumulate into a
            # single pp-tag PSUM group, then one copy to SBUF + store.
            def out_proj_parcel(m, eo):
                po = ppool.tile([128, NQ], F32, tag="pp", name="po")
                for kp in range(2):
                    nc.tensor.matmul(
                        po, ctxT[kp][:, m * 128:(m + 1) * 128],
                        wo_sb[:, kp, eo * NQ:(eo + 1) * NQ],
                        start=(kp == 0), stop=(kp == 1))
                o = opool.tile([128, NQ], F16, tag="o", name="o")
                nc.vector.tensor_copy(o, po)
                nc.gpsimd.dma_start(
                    out=out_part[m * 128:(m + 1) * 128, eo * NQ:(eo + 1) * NQ],
                    in_=o)

            # ---- attention (per head-pair p, nq chunk j of 512) ----
            # Software-pipelined: ctx matmuls for work item u are emitted
            # after the S/exp of item u+1, so PE always has ready work while
            # ACT streams 1024-wide exps; heads alternate as the natural PSUM
            # ping-pong for the S tiles. Filler parcels (projections, wo DMA,
            # out-proj) are popped between work items, paced to spread over
            # the remaining j windows.
            from collections import deque
            fillers = deque()

            for p in range(2):
                if p == 0:
                    fillers.extend(q_rest)
                    fillers.append(lambda: nc.sync.dma_start(
                        out=wo_sb,
                        in_=woT.rearrange("(k p) e -> p k e", p=128)))
                    nf, _ = proj_fillers(1)
                    fillers.extend(nf)
                for j in range(NJA):
                    cps = [cpool.tile([HD + 1, NQA], F32, tag=f"c{h}",
                                      name=f"c{h}") for h in range(2)]

                    def s_mms(g, h):
                        lo = 64 * h
                        sp = spool.tile([128, GK, NQA], F32,
                                        tag=f"s{h}", name=f"s{h}")
                        for i, t in enumerate(g):
                            nc.tensor.matmul(
                                sp[:, i, :],
                                kT[p][lo:lo + 64, t * 128:(t + 1) * 128],
                                qT[p][lo:lo + 64, j * NQA:(j + 1) * NQA],
                                start=True, stop=True)
                        return sp

                    def exp_act(sp, g, h):
                        e = epool.tile([128, GK, NQA], F16,
                                       tag=f"e{h}", name=f"e{h}")
                        nc.scalar.activation(e, sp, FP.Exp, scale=SCALE)
                        return e

                    def ctx_mms(e, g, h):
                        hloc = 2 * p + h
                        for i, t in enumerate(g):
                            nc.tensor.matmul(
                                cps[h],
                                v_all[:, t, hloc * 65:(hloc + 1) * 65],
                                e[:, i, :],
                                start=(t == 0), stop=(t == NT - 1))

                    work = []
                    for gi in range(NG):
                        for h in range(2):
                            work.append((tuple(range(gi * GK, (gi + 1) * GK)), h))
                    n_pop = -(-len(fillers) // (NJA - j))  # ceil
                    prev = None
                    for wi, (g, h) in enumerate(work):
                        # order within an item: S(u) | filler | ctx(u-1) —
                        # the filler between S and ctx gives exp(u-1) the
                        # extra headroom to finish before the in-order PE
                        # queue reaches ctx(u-1), and keeps S(u+1) far enough
                        # behind exp(u-1)'s release of the S-psum tag.
                        sp = s_mms(g, h)
                        cur = (exp_act(sp, g, h), g, h)
                        if fillers and wi < n_pop:
                            fillers.popleft()()
                        if prev is not None:
                            ctx_mms(*prev)
                        prev = cur
                    ctx_mms(*prev)
                    for _ in range(len(work), n_pop):
                        if fillers:
                            fillers.popleft()()

                    # normalize: ctx^T[0:64] * (1 / rowsum); rowsum in row 64.
                    # Two quick DVE copies (rowsum row -> partition 0 for
                    # partition_broadcast, ctx body -> SBUF) release the ctx
                    # PSUM bank ~1.5us after the last ctx matmul so the next
                    # j's ctx accumulation never stalls on it; the slow-ish
                    # broadcast + reciprocal then run off the critical path.
                    for h in range(2):
                        rs = npool.tile([1, NQA], F32, tag="rs", name="rs")
                        nc.vector.tensor_copy(rs, cps[h][64:65, :])
                        cs = npool.tile([64, NQA], F32, tag="cs", name="cs")
                        nc.vector.tensor_copy(cs, cps[h][0:64, :])
                        rb = npool.tile([64, NQA], F32, tag="rb", name="rb")
                        nc.gpsimd.partition_broadcast(rb, rs)
                        rc = npool.tile([64, NQA], F32, tag="rc", name="rc")
                        nc.vector.reciprocal_approx_fast(out=rc, in_=rb)
                        nc.vector.tensor_tensor(
                            out=ctxT[p][h * 64:(h + 1) * 64,
                                        j * NQA:(j + 1) * NQA],
                            in0=cs, in1=rc, op=mult)
                    if p == 1:
                        # ctxT1 columns for this j are final -> out-proj
                        # parcels for the covered m-chunks can run
                        for m in range(4 * j, 4 * j + 4):
                            for eo in range(2):
                                fillers.append(
                                    lambda m=m, eo=eo: out_proj_parcel(m, eo))
            while fillers:
                fillers.popleft()()

    nc.compile()
    return nc


_NC_CACHE = {}


def _get_program():
    if "nc" not in _NC_CACHE:
        _NC_CACHE["nc"] = build_program()
    return _NC_CACHE["nc"]


def make_in_maps(x, Wq, bq, Wk, bk, Wv, bv, Wo, bo):
    x = np.asarray(x)
    xTs = [np.ascontiguousarray(x[b].T.astype(np.float16)) for b in range(B)]
    in_maps = []
    for c in range(N_CORES):
        b, hg = divmod(c, TP)
        sl = slice(hg * DQ, (hg + 1) * DQ)
        in_maps.append({
            "xT": xTs[b],
            "wqT": np.ascontiguousarray(np.asarray(Wq, np.float16)[sl, :].T),
            "wkT": np.ascontiguousarray(np.asarray(Wk, np.float16)[sl, :].T),
            "wvT": np.ascontiguousarray(np.asarray(Wv, np.float16)[sl, :].T),
            "woT": np.ascontiguousarray(np.asarray(Wo, np.float16)[:, sl].T),
            "bq_s": np.ascontiguousarray(np.asarray(bq, np.float32)[sl]),
            "bk_s": np.ascontiguousarray(np.asarray(bk, np.float32)[sl]),
        })
    return in_maps


def assemble_output(results, Wv_bias_term):
    out = np.empty((B, N, EMB), np.float32)
    for b in range(B):
        acc = results[b * TP]["out_part"].astype(np.float32)
        for g in range(1, TP):
            acc = acc + results[b * TP + g]["out_part"]
        out[b] = acc + Wv_bias_term
    return out


def kernel(x, Wq, bq, Wk, bk, Wv, bv, Wo, bo):
    nc = _get_program()
    in_maps = make_in_maps(x, Wq, bq, Wk, bk, Wv, bv, Wo, bo)
    res = run_bass_kernel_spmd(nc, in_maps, list(range(N_CORES)))
    bias_term = (np.asarray(bv, np.float32) @ np.asarray(Wo, np.float32).T
                 + np.asarray(bo, np.float32))
    return assemble_output(res.results, bias_term)
